# revision 12
# baseline (speedup 1.0000x reference)
"""Trainium2 Bass kernel for per-sample masked conv2d (dynamic weight attention conv).

out[b] = conv2d(x[b], weight * m[b], stride=1, pad=1) + bias

Strategy: pure data parallel over batch (32 samples -> 8 cores x 4 samples).
Per sample, the conv is computed as 9 shifted matmuls accumulated in PSUM:
  out[o, h, w] = sum_{kh,kw,i} mw[o,i,kh,kw] * xpad[i, h+kh, w+kw]
with mw = weight * m[b].  The masked weights are produced in natural [o, (i kh kw)]
layout by a DVE elementwise multiply, then transposed 128x128-tile-wise on the
TensorEngine into the [i, o] layout the matmul's stationary operand needs.
Matmuls run as float32r (full-rate fp32 path, N=448 >= 256).
"""

import sys
from contextlib import ExitStack

for _p in ("/opt/trn_rl_repo",):
    if _p not in sys.path:
        sys.path.append(_p)

import numpy as np

import concourse.bass as bass
import concourse.mybir as mybir
import concourse.tile as tile
from concourse import bacc, bass_utils
from concourse.masks import make_identity

# Problem constants (hardcoded per contract)
B, FIN, FOUT, KK, H, W = 32, 256, 256, 3, 56, 56
N_CORES = 8
BPC = B // N_CORES          # samples per core = 4
P = 128                     # partition width
NI = FIN // P               # input-channel chunks = 2
NO = FOUT // P              # output-channel chunks = 2
HP, WP = H + 2, W + 2       # padded spatial = 58x58
RG_ROWS = 8                 # output rows per matmul group
NRG = H // RG_ROWS          # row groups = 7
NTILE = RG_ROWS * W         # moving free size = 448 (<=512 fp32, >=256 for f32r)
F32 = mybir.dt.float32
F32R = mybir.dt.float32r


def build_program():
    """Build the single-core Bass program (same program on all 8 cores)."""
    nc = bacc.Bacc("TRN2", target_bir_lowering=False, debug=False,
                   num_devices=N_CORES)

    x_d = nc.dram_tensor("x", [BPC, FIN, H, W], F32, kind="ExternalInput").ap()
    m_d = nc.dram_tensor("m", [BPC, FOUT, FIN, KK, KK], F32,
                         kind="ExternalInput").ap()
    w_d = nc.dram_tensor("weight", [FOUT, FIN, KK, KK], F32,
                         kind="ExternalInput").ap()
    b_d = nc.dram_tensor("bias", [FOUT], F32, kind="ExternalInput").ap()
    o_d = nc.dram_tensor("out", [BPC, FOUT, H, W], F32,
                         kind="ExternalOutput").ap()

    KSQ = KK * KK                      # 9
    CFREE = FIN * KSQ                  # 2304: (i kh kw) flattened

    with tile.TileContext(nc) as tc, ExitStack() as ctx:
        consts = ctx.enter_context(tc.tile_pool(name="consts", bufs=1))
        m_pool = ctx.enter_context(tc.tile_pool(name="m_pool", bufs=2 * NO))
        xs_pool = ctx.enter_context(tc.tile_pool(name="xs_pool", bufs=2))
        xp_pool = ctx.enter_context(tc.tile_pool(name="xp_pool", bufs=2 * NI))
        wt_pool = ctx.enter_context(tc.tile_pool(name="wt_pool",
                                                 bufs=NO * NI * KSQ))
        out_pool = ctx.enter_context(tc.tile_pool(name="out_pool", bufs=2))
        acc_psum = ctx.enter_context(tc.tile_pool(name="acc_psum", bufs=4,
                                                  space="PSUM"))
        tp_psum = ctx.enter_context(tc.tile_pool(name="tp_psum", bufs=4,
                                                 space="PSUM"))

        # --- per-core constants ---
        ident = consts.tile([P, P], F32, name="ident")
        make_identity(nc, ident)

        # weight in natural layout: [o_chunk][128, (i kh kw)]
        w_nat = w_d.rearrange("(c p) i kh kw -> c p (i kh kw)", p=P)
        w_tiles = []
        for oc in range(NO):
            wt = consts.tile([P, CFREE], F32, name=f"w_nat_{oc}", tag=f"w{oc}")
            nc.sync.dma_start(out=wt, in_=w_nat[oc])
            w_tiles.append(wt)

        # bias: [128, NO] with bias_t[p, oc] = bias[oc*128 + p]
        bias_t = consts.tile([P, NO], F32, name="bias_t")
        nc.sync.dma_start(out=bias_t, in_=b_d.rearrange("(c p) -> p c", p=P))

        x_nat = x_d.rearrange("s (c p) h w -> s c p h w", p=P)
        m_nat = m_d.rearrange("s (c p) i kh kw -> s c p (i kh kw)", p=P)
        o_nat = o_d.rearrange("s (c p) h w -> s c p (h w)", p=P)

        for s in range(BPC):
            # --- masked weights in natural layout, in place over the m tiles ---
            mw_tiles = []
            for oc in range(NO):
                mt = m_pool.tile([P, CFREE], F32, name=f"mw_{s}_{oc}", tag="m")
                nc.sync.dma_start(out=mt, in_=m_nat[s, oc])
                nc.vector.tensor_mul(mt, mt, w_tiles[oc])
                mw_tiles.append(mt)

            # --- padded input tiles [128, 58, 58] in f32r ---
            # fp32r matmul operands must come from a rounding compute op, so
            # DMA contiguously into a staging tile (efficient descriptors),
            # then repack+round into the padded tile on DVE.
            xp_tiles = []
            for ic in range(NI):
                # staging tile carries a 64-elem zero scratch at the end;
                # all xp writes are DVE copies (memset can't emit f32r).
                xs = xs_pool.tile([P, H * W + 64], F32, name=f"xs_{s}_{ic}",
                                  tag="xs")
                nc.vector.memset(xs[:, H * W:], 0.0)
                nc.sync.dma_start(out=xs[:, :H * W], in_=x_nat[s, ic])
                xp = xp_pool.tile([P, HP, WP], F32R, name=f"xp_{s}_{ic}",
                                  tag="xp")
                z = xs[:, H * W:H * W + WP]
                nc.vector.tensor_copy(xp[:, 0, :], z)
                nc.vector.tensor_copy(xp[:, HP - 1, :], z)
                zc = xs[:, H * W:H * W + H].rearrange("p (h o) -> p h o", o=1)
                nc.vector.tensor_copy(xp[:, 1:HP - 1, 0:1], zc)
                nc.vector.tensor_copy(xp[:, 1:HP - 1, WP - 1:WP], zc)
                nc.vector.tensor_copy(
                    xp[:, 1:HP - 1, 1:WP - 1],
                    xs[:, :H * W].rearrange("p (h w) -> p h w", w=W))
                xp_tiles.append(xp)

            # --- transpose masked weights into [i, o] stationary tiles ---
            # mwT[oc][ic][k][i_part, o_free] = mw[o, i, kh, kw]
            mwT = [[[None] * KSQ for _ in range(NI)] for _ in range(NO)]
            for oc in range(NO):
                mw3 = mw_tiles[oc].rearrange("p (i k) -> p i k", k=KSQ)
                for ic in range(NI):
                    for k in range(KSQ):
                        tp = tp_psum.tile([P, P], F32, name=f"tp_{s}_{oc}_{ic}_{k}",
                                          tag="tp")
                        nc.tensor.transpose(tp, mw3[:, ic * P:(ic + 1) * P, k],
                                            ident)
                        wt = wt_pool.tile([P, P], F32R,
                                          name=f"mwT_{s}_{oc}_{ic}_{k}", tag="mwT")
                        nc.vector.tensor_copy(wt, tp)
                        mwT[oc][ic][k] = wt

            # --- conv matmuls ---
            for oc in range(NO):
                osb = out_pool.tile([P, H * W], F32, name=f"osb_{s}_{oc}",
                                    tag="osb")
                for rg in range(NRG):
                    acc = acc_psum.tile([P, NTILE], F32, name=f"acc_{s}_{oc}_{rg}",
                                        tag="acc")
                    n_mm = KSQ * NI
                    idx = 0
                    for k in range(KSQ):
                        kh, kw = divmod(k, KK)
                        r0 = rg * RG_ROWS + kh
                        for ic in range(NI):
                            rhs = xp_tiles[ic][:, r0:r0 + RG_ROWS, kw:kw + W]
                            nc.tensor.matmul(
                                acc,
                                mwT[oc][ic][k],
                                rhs,
                                start=(idx == 0),
                                stop=(idx == n_mm - 1),
                            )
                            idx += 1
                    # drain PSUM -> SBUF with bias add (ACT: out = in * 1 + bias)
                    nc.scalar.add(osb[:, rg * NTILE:(rg + 1) * NTILE], acc,
                                  bias_t[:, oc:oc + 1])
                nc.sync.dma_start(out=o_nat[s, oc], in_=osb)

    nc.compile()
    return nc


def shard_inputs(x, m, weight, bias):
    """Split batch across cores; replicate weight/bias."""
    x = np.ascontiguousarray(np.asarray(x, dtype=np.float32))
    m = np.ascontiguousarray(np.asarray(m, dtype=np.float32))
    weight = np.ascontiguousarray(np.asarray(weight, dtype=np.float32))
    bias = np.ascontiguousarray(np.asarray(bias, dtype=np.float32))
    in_maps = []
    for c in range(N_CORES):
        sl = slice(c * BPC, (c + 1) * BPC)
        in_maps.append({"x": x[sl], "m": m[sl], "weight": weight, "bias": bias})
    return in_maps


def kernel(x, m, weight, bias, _trace=False):
    nc = build_program()
    in_maps = shard_inputs(x, m, weight, bias)
    res = bass_utils.run_bass_kernel_spmd(
        nc, in_maps, core_ids=list(range(N_CORES)), trace=_trace
    )
    out = np.concatenate([res.results[c]["out"] for c in range(N_CORES)], axis=0)
    if _trace:
        kernel.last_results = res
    return out
